# revision 1
# baseline (speedup 1.0000x reference)
"""Trainium2 Bass kernel for the CPCA auxiliary loss (nn_CPCA_51754355917033).

Strategy (data-parallel over the env/batch dim n, 16 envs per core):
  - Host side (sharding prep): every gather in the reference is baked into
    per-core contiguous device inputs -- action-embedding lookup, h0 gather
    from rnn_outputs, target gather from rnn_inputs, the 20480-row negative
    gather, and the not_dones gather (raw values, zero padded).  All matmul
    operands are laid out pre-transposed (contraction dim on partitions) and
    converted to bf16.
  - Device side (all the FP math): GRU unrolled u_max+1 steps in transposed
    layout, MLP over 21 "blocks" (20 negative g-blocks + 1 positive block,
    identical code path), masked-loss partial sums per core.
  - Host combines the 8 cores' (pos_sum, neg_sum, denom) partials into the
    final scalar (the all-reduce of the sharding hint, done at unshard time).
"""

import numpy as np
import ml_dtypes

import concourse.bass as bass
import concourse.mybir as mybir
import concourse.tile as tile
from concourse import bacc
from concourse import bass_utils

BF16 = ml_dtypes.bfloat16
F8 = ml_dtypes.float8_e4m3
DT = mybir.dt
AF = mybir.ActivationFunctionType
ALU = mybir.AluOpType

N, T, H, K, S, F, EMB, NLOG, NEG = 128, 512, 512, 16, 16, 4, 32, 18, 20
COEFF = 0.1
NC = 8
NPC = N // NC          # envs per core
R = NPC * S            # GRU rows per core (256)
L = T - 1
NBLK = NEG + 1         # 20 negative g-blocks + 1 positive block
BR = F * R             # rows per block (1024)

_PROGRAM_CACHE = {}
_USE_TTR = False  # InstTensorTensorReduce fails on HW via this exec path


# ----------------------------------------------------------------- host prep

def _prep_core(c, inputs, u_list, k_eff):
    acts = np.asarray(inputs["actions"])[..., 0]
    nd = np.asarray(inputs["not_dones"])[..., 0]
    ri = np.asarray(inputs["rnn_inputs"], np.float32)
    ro = np.asarray(inputs["rnn_outputs"], np.float32)
    ti = np.asarray(inputs["time_subsample"]).astype(np.int64)
    neg_idx = np.asarray(inputs["neg_idx"]).astype(np.int64)
    emb_tab = np.asarray(inputs["action_embed"], np.float32)

    ns = slice(c * NPC, (c + 1) * NPC)
    idx = np.arange(k_eff)[:, None] + ti[None, :]          # (k_eff, S)

    # gi = emb @ W_ih.T folded on host: a gather from the 18-row table
    # GIE = action_embed @ W_ih.T (weight preprocessing), zero for padding.
    W_ih = np.asarray(inputs["W_ih"], np.float32)
    b_ih = np.asarray(inputs["b_ih"], np.float32)
    GIE = np.zeros((NLOG + 1, 1536), np.float32)
    GIE[:NLOG] = emb_tab @ W_ih.T + b_ih
    GIE[NLOG] = b_ih
    act_ext = np.full((NPC, L + K), NLOG, np.int64)
    act_ext[:, :L] = acts[ns, :L]
    AI = act_ext[:, idx]                                   # (NPC, k_eff, S)
    gi_all = GIE[AI.transpose(1, 0, 2).reshape(k_eff, R)]  # (k_eff, R, 1536)
    giT = np.ascontiguousarray(
        gi_all.transpose(0, 2, 1).reshape(k_eff, 12, 128, R)
        .transpose(0, 2, 1, 3)).astype(BF16)               # (k_eff,128,12,R)

    H0 = ro[ns][:, ti]                                     # (NPC, S, H)
    h0T = np.ascontiguousarray(
        H0.transpose(2, 0, 1).reshape(4, 128, R)).astype(BF16)

    ri_ext = np.zeros((NPC, L + K, H), np.float32)
    ri_ext[:, :L] = ri[ns, 1:]
    idx2 = np.asarray(u_list)[:, None] + ti[None, :]       # (F, S)
    TG = ri_ext[:, idx2]                                   # (NPC, F, S, H)
    tgT = np.ascontiguousarray(
        TG.transpose(3, 1, 0, 2).reshape(H, BR).reshape(4, 128, BR)).astype(F8)

    ni = neg_idx.reshape(F, N, S, NEG)[:, ns]              # (F, NPC, S, NEG)
    P = ni.transpose(3, 0, 1, 2).reshape(-1)               # cols in (g, f, j) order
    negs = ri.reshape(N * T, H)[P]
    negsT = np.ascontiguousarray(negs.T.reshape(4, 128, NEG * BR)).astype(F8)

    nd_ext = np.zeros((NPC, L + K), np.float32)
    nd_ext[:, :L] = nd[ns, :L]
    G = nd_ext[:, idx]                                     # (NPC, k_eff, S)
    ndv = G.transpose(1, 0, 2).reshape(k_eff, R)
    ndvT = np.ascontiguousarray(
        ndv.reshape(k_eff, 2, 128).transpose(2, 0, 1)).astype(np.float32)

    return dict(giT=giT, h0T=h0T, tgT=tgT, negsT=negsT, ndvT=ndvT)


def _prep_weights(inputs):
    W_ih = np.asarray(inputs["W_ih"], np.float32)
    W_hh = np.asarray(inputs["W_hh"], np.float32)
    b_ih = np.asarray(inputs["b_ih"], np.float32)
    b_hh = np.asarray(inputs["b_hh"], np.float32)
    W1 = np.asarray(inputs["W1"], np.float32)
    b1 = np.asarray(inputs["b1"], np.float32)
    W2 = np.asarray(inputs["W2"], np.float32)
    b2 = np.asarray(inputs["b2"], np.float32)
    W3 = np.asarray(inputs["W3"], np.float32)
    b3 = np.asarray(inputs["b3"], np.float32)

    d = {}
    d["w_hh8"] = np.ascontiguousarray(
        W_hh.T.reshape(2, 2, 128, 1536).transpose(0, 2, 1, 3)).astype(F8)
    def pack8(WT):
        # [t, ki, ko, m] with contract index = t*256 + ko*128 + ki
        return np.ascontiguousarray(
            WT.reshape(2, 2, 128, WT.shape[1]).transpose(0, 2, 1, 3)).astype(F8)
    d["w1a8"] = pack8(W1[:, :512].T.copy())
    d["w1b8"] = pack8(W1[:, 512:].T.copy())
    d["w28"] = pack8(W2.T.copy())
    d["w3T"] = np.ascontiguousarray(W3[0].reshape(4, 128).T).astype(BF16)
    bg = np.zeros((128, 16), np.float32)
    for cc in range(12):
        bg[:, cc] = b_hh[cc * 128:(cc + 1) * 128]
    d["bgates"] = bg
    d["b1T"] = np.ascontiguousarray(b1.reshape(4, 128).T).astype(np.float32)
    d["b2T"] = np.ascontiguousarray(b2.reshape(4, 128).T).astype(np.float32)
    d["b3f"] = float(b3.reshape(-1)[0])
    return d


# ------------------------------------------------------------- device program

def _build_program(u_list, k_eff, b3f, upto=4):
    nc = bacc.Bacc("TRN2", target_bir_lowering=False, debug=False, num_devices=NC)

    di = {}
    def inp(name, shape, dt):
        di[name] = nc.dram_tensor(name, list(shape), dt, kind="ExternalInput")
        return di[name]

    d_whh = inp("w_hh8", (2, 128, 2, 1536), DT.float8e4)
    d_w1a = inp("w1a8", (2, 128, 2, 512), DT.float8e4)
    d_w1b = inp("w1b8", (2, 128, 2, 512), DT.float8e4)
    d_w2 = inp("w28", (2, 128, 2, 512), DT.float8e4)
    d_w3 = inp("w3T", (128, 4), DT.bfloat16)
    d_bg = inp("bgates", (128, 16), DT.float32)
    d_b1 = inp("b1T", (128, 4), DT.float32)
    d_b2 = inp("b2T", (128, 4), DT.float32)
    d_gi = inp("giT", (k_eff, 128, 12, R), DT.bfloat16)
    d_h0 = inp("h0T", (4, 128, R), DT.bfloat16)
    d_tg = inp("tgT", (4, 128, BR), DT.float8e4)
    d_negs = inp("negsT", (4, 128, NEG * BR), DT.float8e4)
    d_ndv = inp("ndvT", (128, k_eff, 2), DT.float32)
    d_out = nc.dram_tensor("out", [1, 4], DT.float32, kind="ExternalOutput")

    f32 = DT.float32
    bf16 = DT.bfloat16

    with tile.TileContext(nc) as tc:
        with (
            tc.tile_pool(name="const", bufs=1) as cp,
            tc.tile_pool(name="gruw", bufs=2) as gp,
            tc.tile_pool(name="mlpw", bufs=3) as mp,
            tc.tile_pool(name="psg", bufs=3, space="PSUM") as pg,
            tc.tile_pool(name="psm", bufs=2, space="PSUM") as pm,
            tc.tile_pool(name="psa", bufs=1, space="PSUM") as pa,
        ):
            # ------------------------------------------------ constant loads
            f8 = DT.float8e4
            whh = cp.tile([128, 2, 2, 1536], f8, tag="whh")
            for th in range(2):
                nc.sync.dma_start(out=whh[:, th, :, :], in_=d_whh[th])
            w1a = cp.tile([128, 2, 2, 512], f8, tag="w1a")
            w1b = cp.tile([128, 2, 2, 512], f8, tag="w1b")
            w2 = cp.tile([128, 2, 2, 512], f8, tag="w2")
            for (t, d) in ((w1a, d_w1a), (w1b, d_w1b), (w2, d_w2)):
                for th in range(2):
                    nc.sync.dma_start(out=t[:, th, :, :], in_=d[th])
            w3 = cp.tile([128, 4], bf16, tag="w3")
            nc.sync.dma_start(out=w3[:], in_=d_w3[:])
            bg = cp.tile([128, 16], f32, tag="bg")
            nc.sync.dma_start(out=bg[:], in_=d_bg[:])
            b1 = cp.tile([128, 4], f32, tag="b1")
            nc.sync.dma_start(out=b1[:], in_=d_b1[:])
            b2 = cp.tile([128, 4], f32, tag="b2")
            nc.sync.dma_start(out=b2[:], in_=d_b2[:])
            tg = cp.tile([128, 4, BR], f8, tag="tg")
            for kc in range(4):
                nc.sync.dma_start(out=tg[:, kc, :], in_=d_tg[kc])
            ndv = cp.tile([128, k_eff, 2], f32, tag="ndv")
            nc.sync.dma_start(out=ndv[:], in_=d_ndv[:])

            # ------------------------------------------------ forward mask
            prod = cp.tile([128, k_eff, 2], f32, tag="prod")
            nc.vector.tensor_scalar(prod[:, 0, :], ndv[:, 0, :], 0.0, None,
                                    op0=ALU.is_gt)
            for k in range(1, k_eff):
                nc.vector.scalar_tensor_tensor(
                    prod[:, k, :], in0=ndv[:, k, :], scalar=0.0,
                    in1=prod[:, k - 1, :], op0=ALU.is_gt, op1=ALU.mult)
            mfT = cp.tile([128, 2 * F], f32, tag="mfT")
            for fi, u in enumerate(u_list):
                nc.vector.tensor_copy(mfT[:, 2 * fi:2 * fi + 2], prod[:, u, :])

            # ------------------------------------------------ GRU
            # h state kept in bf16. Gate tail is per-chunk pipelined and
            # z*h / (1-z) are computed off the critical path so the next
            # step's matmuls start ~2us after the last ig matmul.
            do_gru = upto >= 1
            h_prev = gp.tile([128, 4, R], bf16, tag="h")
            for kc in range(4):
                nc.sync.dma_start(out=h_prev[:, kc, :], in_=d_h0[kc])
            h8_prev = gp.tile([128, 4, R], f8, tag="h8")
            nc.vector.tensor_copy(h8_prev[:], h_prev[:])
            predsT = cp.tile([128, 4, BR], f8, tag="preds")

            DRM = mybir.MatmulPerfMode.DoubleRow
            for k in range(k_eff if do_gru else 0):
                gi = gp.tile([128, 12, R], bf16, tag="gi", bufs=3)
                nc.sync.dma_start(out=gi[:], in_=d_gi[k])
                rzt = gp.tile([128, 8, R], f32, tag="rzt", bufs=1)
                r_sb = gp.tile([128, 4, R], f32, tag="r")
                z_sb = gp.tile([128, 4, R], f32, tag="z")
                e_sb = gp.tile([128, 4, R], f32, tag="e")
                w1m = gp.tile([128, 4, R], f32, tag="w1m")
                t_sb = gp.tile([128, 4, R], f32, tag="t", bufs=1)
                u_sb = gp.tile([128, 4, R], f32, tag="u", bufs=1)
                g_sb = gp.tile([128, 4, R], f32, tag="g")
                gw = gp.tile([128, 4, R], f32, tag="gw", bufs=1)
                h_new = gp.tile([128, 4, R], bf16, tag="h")
                h8_new = gp.tile([128, 4, R], f8, tag="h8")

                for gc in range(8):
                    ps = pg.tile([128, R], f32, tag="pg")
                    for th in range(2):
                        nc.tensor.matmul(
                            ps[:], whh[:, th, :, gc * 128:(gc + 1) * 128],
                            h8_prev[:, 2 * th:2 * th + 2, :],
                            start=(th == 0), stop=(th == 1), perf_mode=DRM)
                    nc.vector.scalar_tensor_tensor(
                        rzt[:, gc, :], in0=ps[:], scalar=bg[:, gc:gc + 1],
                        in1=gi[:, gc, :], op0=ALU.add, op1=ALU.add)
                nc.scalar.activation(r_sb[:], rzt[:, 0:4, :], AF.Sigmoid)
                nc.scalar.activation(z_sb[:], rzt[:, 4:8, :], AF.Sigmoid)
                nc.vector.tensor_mul(e_sb[:], z_sb[:], h_prev[:])
                nc.scalar.activation(w1m[:], z_sb[:], AF.Identity,
                                     scale=-1.0, bias=1.0)
                for c in range(4):
                    ps = pg.tile([128, R], f32, tag="pg")
                    gc = 8 + c
                    for th in range(2):
                        nc.tensor.matmul(
                            ps[:], whh[:, th, :, gc * 128:(gc + 1) * 128],
                            h8_prev[:, 2 * th:2 * th + 2, :],
                            start=(th == 0), stop=(th == 1), perf_mode=DRM)
                    nc.vector.scalar_tensor_tensor(
                        t_sb[:, c, :], in0=ps[:], scalar=bg[:, 8 + c:9 + c],
                        in1=r_sb[:, c, :], op0=ALU.add, op1=ALU.mult)
                    nc.vector.tensor_add(u_sb[:, c, :], gi[:, 8 + c, :],
                                         t_sb[:, c, :])
                    nc.scalar.activation(g_sb[:, c, :], u_sb[:, c, :], AF.Tanh)
                    nc.vector.tensor_mul(gw[:, c, :], g_sb[:, c, :],
                                         w1m[:, c, :])
                    nc.vector.tensor_add(h_new[:, c, :], gw[:, c, :],
                                         e_sb[:, c, :])
                    nc.vector.tensor_copy(h8_new[:, c, :], h_new[:, c, :])
                h_prev = h_new
                h8_prev = h8_new
                for fi, u in enumerate(u_list):
                    if u == k:
                        nc.vector.tensor_copy(
                            predsT[:, :, fi * R:(fi + 1) * R], h8_new[:])

            # ------------------------------------------------ blocks
            # L1/L2 run in fp8e4 with DoubleRow (2 contraction chunks per
            # matmul); the preds@W1a part accumulates into the same PSUM
            # group, so eviction is a single Relu with the b1 bias.
            DR = mybir.MatmulPerfMode.DoubleRow
            logits = cp.tile([128, NBLK, 8], f32, tag="logits")
            for b in range(NBLK if upto >= 3 else 0):
                if b < NEG:
                    xt = mp.tile([128, 4, BR], f8, tag="negsx")
                    for kc in range(4):
                        nc.sync.dma_start(
                            out=xt[:, kc, :],
                            in_=d_negs[kc][:, b * BR:(b + 1) * BR])
                else:
                    xt = tg
                y1 = mp.tile([128, 4, BR], f8, tag="y1", bufs=2)
                for cc in range(4):
                    ps = pm.tile([128, 2, 512], f32, tag="pm")
                    for rt in range(2):
                        sl = slice(rt * 512, (rt + 1) * 512)
                        for th in range(2):
                            nc.tensor.matmul(
                                ps[:, rt, :],
                                w1b[:, th, :, cc * 128:(cc + 1) * 128],
                                xt[:, 2 * th:2 * th + 2, sl],
                                start=(th == 0), stop=False, perf_mode=DR)
                        for th in range(2):
                            nc.tensor.matmul(
                                ps[:, rt, :],
                                w1a[:, th, :, cc * 128:(cc + 1) * 128],
                                predsT[:, 2 * th:2 * th + 2, sl],
                                start=False, stop=(th == 1), perf_mode=DR)
                    nc.vector.tensor_scalar(y1[:, cc, :], ps[:],
                                            b1[:, cc:cc + 1], 0.0,
                                            op0=ALU.add, op1=ALU.max)
                y2 = mp.tile([128, 4, BR], bf16, tag="y2", bufs=2)
                for cc in range(4):
                    ps = pm.tile([128, 2, 512], f32, tag="pm")
                    for rt in range(2):
                        sl = slice(rt * 512, (rt + 1) * 512)
                        for th in range(2):
                            nc.tensor.matmul(
                                ps[:, rt, :],
                                w2[:, th, :, cc * 128:(cc + 1) * 128],
                                y1[:, 2 * th:2 * th + 2, sl],
                                start=(th == 0), stop=(th == 1), perf_mode=DR)
                    nc.scalar.activation(y2[:, cc, :], ps[:], AF.Relu,
                                         bias=b2[:, cc:cc + 1])
                ps3 = pa.tile([128, 8], f32, tag="pa3")
                for col in range(8):
                    for kc in range(4):
                        nc.tensor.matmul(
                            ps3[:, col:col + 1],
                            y2[:, kc, col * 128:(col + 1) * 128],
                            w3[:, kc:kc + 1], start=(kc == 0), stop=(kc == 3))
                nc.scalar.activation(logits[:, b, :], ps3[:], AF.Copy)

            # ------------------------------------- softplus + sums
            # softplus(t) = relu(t) - ln(sigmoid(|t|)); whole-tensor ACT ops
            # keep the activation-table sequence to a single switch.
            partials = cp.tile([128, NBLK + 1], f32, tag="partials")
            sp_a = cp.tile([128, NBLK, 8], f32, tag="sp_a")
            sp_l = cp.tile([128, NBLK, 8], f32, tag="sp_l")
            sp_r = cp.tile([128, NBLK, 8], f32, tag="sp_r")
            sp_d = cp.tile([128, 8], f32, tag="sp_d")
            if upto >= 4:
                nc.scalar.activation(sp_a[:], logits[:], AF.Abs, bias=b3f)
                nc.scalar.activation(sp_a[:], sp_a[:], AF.Sigmoid)
                nc.scalar.activation(sp_l[:], sp_a[:], AF.Ln)
                nc.scalar.activation(sp_r[:, :NEG, :], logits[:, :NEG, :],
                                     AF.Relu, bias=b3f)
                nc.scalar.activation(sp_r[:, NEG, :], logits[:, NEG, :],
                                     AF.Relu, bias=-b3f, scale=-1.0)
                nc.vector.tensor_sub(sp_r[:], sp_r[:], sp_l[:])
                for b in range(NBLK):
                    nc.vector.tensor_mul(sp_d[:], sp_r[:, b, :], mfT[:])
                    nc.vector.tensor_reduce(partials[:, b:b + 1], sp_d[:],
                                            mybir.AxisListType.X, ALU.add)
            nc.vector.tensor_reduce(partials[:, NBLK:NBLK + 1], mfT[:],
                                    mybir.AxisListType.X, ALU.add)

            vcol = cp.tile([128, 4], f32, tag="vcol")
            nc.vector.tensor_copy(vcol[:, 0:1], partials[:, NEG:NEG + 1])
            nc.vector.tensor_reduce(vcol[:, 1:2], partials[:, 0:NEG],
                                    mybir.AxisListType.X, ALU.add)
            nc.vector.tensor_copy(vcol[:, 2:3], partials[:, NBLK:NBLK + 1])
            nc.any.memset(vcol[:, 3:4], 0.0)
            ones = cp.tile([128, 1], f32, tag="ones")
            nc.any.memset(ones[:], 1.0)
            psf = pa.tile([1, 4], f32, tag="pa3")
            nc.tensor.matmul(psf[:], ones[:], vcol[:], start=True, stop=True)
            out_sb = cp.tile([1, 4], f32, tag="out_sb")
            nc.scalar.activation(out_sb[:], psf[:], AF.Copy)
            nc.sync.dma_start(out=d_out[:], in_=out_sb[:])

    nc.finalize()
    return nc


def _get_program(u_list, k_eff, b3f):
    key = (tuple(u_list), k_eff, float(b3f))
    if key not in _PROGRAM_CACHE:
        _PROGRAM_CACHE[key] = _build_program(u_list, k_eff, b3f)
    return _PROGRAM_CACHE[key]


# ------------------------------------------------------------------ kernel

def kernel(**inputs):
    u_list = [int(x) for x in np.asarray(inputs["unroll_subsample"]).reshape(-1)]
    k_eff = max(u_list) + 1
    w = _prep_weights(inputs)
    nc = _get_program(u_list, k_eff, w["b3f"])

    wmaps = {k: v for k, v in w.items() if k != "b3f"}
    in_maps = []
    for c in range(NC):
        m = dict(wmaps)
        m.update(_prep_core(c, inputs, u_list, k_eff))
        in_maps.append(m)

    res = bass_utils.run_bass_kernel_spmd(nc, in_maps, list(range(NC)))
    P = Ng = D = 0.0
    for c in range(NC):
        o = np.asarray(res.results[c]["out"], np.float64)
        P += o[0, 0]
        Ng += o[0, 1]
        D += o[0, 2]
    loss = COEFF * (P / D + Ng / (D * NEG))
    return np.float32(loss)

